# revision 1
# baseline (speedup 1.0000x reference)
import os as _os
import sys as _sys

for _p in ("/opt/trn_rl_repo", "/root/.axon_site/_ro/trn_rl_repo",
           "/root/.axon_site", "/root/.axon_site/_ro/pypackages"):
    if _os.path.isdir(_p) and _p not in _sys.path:
        _sys.path.append(_p)

"""DCNv2 block kernel for TRN2 (Bass/Tile).

Per-core program: one batch sample.
  x [1024, 384] -> transpose -> padded image xTp [384ch, 48*48]
  offset conv 3x3 (384->72) -> offsets -> bilinear sample indices/weights
  ap_gather sampling (d=2 packed x-pairs, bf16) -> weighted combine
  dcn matmul (K=3456) -> BN+SiLU -> 1x1 conv -> transpose -> out [1024, 384]

Host prep reorders/transposes weights only (layout, no math except BN-free).
"""

import numpy as np
from contextlib import ExitStack

import concourse.bass as bass
import concourse.tile as tile
from concourse import mybir
from concourse import library_config

F32 = mybir.dt.float32
BF16 = mybir.dt.bfloat16
I16 = mybir.dt.int16
I32 = mybir.dt.int32
ALU = mybir.AluOpType
ACTF = mybir.ActivationFunctionType

DIM, KK, G, Cg = 384, 9, 4, 96
H = W = 32
HW = 1024
PAD = 7
PH = PW = H + 2 * PAD          # 48
PHW = PH * PW                  # 2304
NT = KK                        # taps
NCT = DIM // 128               # 3 channel tiles (K side)
NM = DIM // 128                # 3 output tiles
OFFC = G * 2 * KK              # 72
OFFP = 100                     # conv-out rows: dy 0..35, dx 64..99 (32-aligned)
XOFF = 64

NPT = HW // 128                # 8 pixel tiles
MAGIC = float(2 ** 23)


def gk_row(g, k):
    return g * KK + k


def host_prep(inputs: dict) -> dict:
    """Pure-layout host prep of weights/constants. Returns dict of named arrays
    shared by all cores."""
    w_off = np.asarray(inputs["w_off"], np.float32)      # [72, 384, 3, 3]
    b_off = np.asarray(inputs["b_off"], np.float32)      # [72]
    w_dcn = np.asarray(inputs["w_dcn"], np.float32)      # [384, 384, 3, 3]
    w2 = np.asarray(inputs["w2"], np.float32)            # [384, 384]

    # --- offset conv weights: reorder out-channels to [dy(36) ; dx(36)],
    # rows gk = g*9+k.  orig channel = g*18 + k*2 + axis
    w_off_p = np.zeros((OFFP, DIM, 3, 3), np.float32)
    b_off_p = np.zeros((36, 2), np.float32)
    for g in range(G):
        for k in range(KK):
            w_off_p[gk_row(g, k)] = w_off[g * 18 + k * 2 + 0]
            w_off_p[XOFF + gk_row(g, k)] = w_off[g * 18 + k * 2 + 1]
            b_off_p[gk_row(g, k), 0] = b_off[g * 18 + k * 2 + 0]
            b_off_p[gk_row(g, k), 1] = b_off[g * 18 + k * 2 + 1]

    # lhsT tiles for offset conv: [128, 27, 100]; K order = (tap, c)
    w_offT = np.zeros((128, NT * NCT, OFFP), np.float32)
    for t in range(NT):
        ky, kx = t // 3, t % 3
        for ct in range(NCT):
            cs = ct * 128
            # lhsT[r, tile, o] = w_off_p[o, cs+r, ky, kx]
            w_offT[:, t * NCT + ct, :] = w_off_p[:, cs:cs + 128, ky, kx].T
    w_offT = w_offT.reshape(128, NT * NCT * OFFP)

    # grid [72, 1024]: pos = off + grid gives padded coords directly.
    # Columns are in rho-order (rho(j) = (j%64)*16 + j//64) to match the
    # offset conv's N order; makes the ap_gather idx wrap a natural reshape.
    jj = np.arange(HW)
    rho = (jj % 64) * 16 + jj // 64
    grid = np.zeros((36, 2 * HW), np.float32)
    yy = (np.arange(HW) // W)[rho]
    xx = (np.arange(HW) % W)[rho]
    for g in range(G):
        for k in range(KK):
            grid[gk_row(g, k), 0:HW] = (k // 3 - 1) + yy + PAD
            grid[gk_row(g, k), HW:] = (k % 3 - 1) + xx + PAD

    # one-hot broadcast matrices [36, 27*128]
    eoh = np.zeros((36, NT * NCT * 128), np.float32)
    for t in range(NT):
        for ct in range(NCT):
            for m in range(128):
                c = ct * 128 + m
                eoh[gk_row(c // Cg, t), (t * NCT + ct) * 128 + m] = 1.0

    # 1x1 lhsT tiles [128, 3, 384]
    w2T = np.zeros((128, NCT, DIM), np.float32)
    for kt in range(NCT):
        w2T[:, kt, :] = w2[:, kt * 128:(kt + 1) * 128].T
    w2T = w2T.reshape(128, NCT * DIM)

    ident = np.eye(128, dtype=np.float32)
    import ml_dtypes
    identb = np.eye(128, dtype=np.float32).astype(ml_dtypes.bfloat16)
    w_dcn_r = w_dcn.reshape(DIM, DIM, KK)
    w_dcnT2 = np.zeros((128, NT * NCT, DIM), np.float32)
    for t in range(NT):
        for ct in range(NCT):
            cs = ct * 128
            w_dcnT2[:, t * NCT + ct, :] = w_dcn_r[:, cs:cs + 128, t].T
    consts = {
        "w_offT": w_offT,
        "b_off_p": b_off_p,
        "grid": grid,
        "eoh": eoh.astype(ml_dtypes.bfloat16),
        "w_dcnT": w_dcnT2.reshape(128, NT * NCT * DIM).astype(ml_dtypes.bfloat16),
        "w2T": w2T.astype(ml_dtypes.bfloat16),
        "ident": ident,
        "ident_bf": identb,
        "bn_gamma": np.asarray(inputs["bn_gamma"], np.float32),
        "bn_beta": np.asarray(inputs["bn_beta"], np.float32),
        "bn_mean": np.asarray(inputs["bn_mean"], np.float32),
        "bn_var": np.asarray(inputs["bn_var"], np.float32),
        "b2": np.asarray(inputs["b2"], np.float32),
    }
    return consts


def declare_io(nc: bass.Bass, consts: dict):
    """Declare DRAM tensors; returns dict name->AP."""
    aps = {}
    aps["x"] = nc.dram_tensor("x", [HW, DIM], F32, kind="ExternalInput").ap()
    for name, arr in consts.items():
        dt = {np.dtype("float32"): F32}.get(arr.dtype, BF16)
        aps[name] = nc.dram_tensor(name, list(arr.shape), dt, kind="ExternalInput").ap()
    aps["out"] = nc.dram_tensor("out", [HW, DIM], F32, kind="ExternalOutput").ap()
    return aps


def build(ctx: ExitStack, tc: tile.TileContext, io: dict):
    nc = tc.nc
    P = 128
    nc.gpsimd.load_library(library_config.ap_gather)

    const_pool = ctx.enter_context(tc.tile_pool(name="consts", bufs=1))
    d2_pool = ctx.enter_context(tc.tile_pool(name="d2", bufs=1))
    samp_pool = ctx.enter_context(tc.tile_pool(name="sampled", bufs=1))
    work_pool = ctx.enter_context(tc.tile_pool(name="work", bufs=2))
    mid_pool = ctx.enter_context(tc.tile_pool(name="mid", bufs=1))
    off_s = mid_pool.tile([36, 2 * HW], F32, name="off_s")
    widx = mid_pool.tile([P, NCT * NT * 64], I16, name="widx")

    # ---------- load constants ----------
    w_offT = const_pool.tile([P, NT * NCT * OFFP], F32)
    nc.sync.dma_start(w_offT[:], io["w_offT"][:])
    grid_s = const_pool.tile([36, 2 * HW], F32)
    nc.sync.dma_start(grid_s[:], io["grid"][:])
    eoh_s = const_pool.tile([36, NT * NCT * 128], BF16)
    nc.sync.dma_start(eoh_s[:], io["eoh"][:])
    w_dcnT = const_pool.tile([P, NT * NCT * DIM], BF16)
    nc.sync.dma_start(w_dcnT[:], io["w_dcnT"][:])
    w2T = const_pool.tile([P, NCT * DIM], BF16)
    nc.sync.dma_start(w2T[:], io["w2T"][:])
    ident = const_pool.tile([P, P], F32)
    nc.sync.dma_start(ident[:], io["ident"][:])
    ident_bf = const_pool.tile([P, P], BF16)
    nc.sync.dma_start(ident_bf[:], io["ident_bf"][:])
    b_off_s = const_pool.tile([36, 2], F32)
    nc.sync.dma_start(b_off_s[:], io["b_off_p"][:])

    # BN / bias vectors per output tile [128,1] x 3
    bnv = {}
    for vname in ("bn_gamma", "bn_beta", "bn_mean", "bn_var", "b2"):
        tl = const_pool.tile([P, NM], F32, tag=f"bn_{vname}", name=f"bn_{vname}")
        for m in range(NM):
            nc.sync.dma_start(
                tl[:, m:m + 1],
                io[vname][:].rearrange("(m p u) -> m p u", p=P, u=1)[m],
            )
        bnv[vname] = tl

    # BN scale/shift [128, m]
    bn_scale = const_pool.tile([P, NM], F32)
    bn_shift = const_pool.tile([P, NM], F32)
    tmpv = const_pool.tile([P, NM], F32)
    nc.vector.tensor_scalar(tmpv[:], bnv["bn_var"][:], 1e-5, None, op0=ALU.add)
    nc.scalar.sqrt(tmpv[:], tmpv[:])
    nc.vector.reciprocal(tmpv[:], tmpv[:])
    nc.vector.tensor_tensor(bn_scale[:], bnv["bn_gamma"][:], tmpv[:], op=ALU.mult)
    nc.vector.tensor_tensor(tmpv[:], bnv["bn_mean"][:], bn_scale[:], op=ALU.mult)
    nc.vector.tensor_tensor(bn_shift[:], bnv["bn_beta"][:], tmpv[:], op=ALU.subtract)

    # ---------- phase 1: load x, transpose into padded image ----------
    xtp_cm = tc.tile_pool(name="xtp", bufs=1)  # freed after offset conv
    xtp_pool = xtp_cm.__enter__()
    xtp = [xtp_pool.tile([P, PHW + PW + 2], F32, tag=f"xtp{ct}", name=f"xtp{ct}") for ct in range(NCT)]
    for ct in range(NCT):
        nc.vector.memset(xtp[ct][:], 0.0)

    with tc.tile_pool(name="ptrans", bufs=4, space="PSUM") as psum_t, \
         tc.tile_pool(name="xin", bufs=3) as xin_pool:
        for pt in range(NPT):
            xin = xin_pool.tile([P, DIM], F32)
            nc.sync.dma_start(xin[:], io["x"][pt * P:(pt + 1) * P, :])
            for ct in range(NCT):
                ps = psum_t.tile([P, P], F32)
                nc.tensor.transpose(ps[:], xin[:, ct * P:(ct + 1) * P], ident[:])
                # write [128c, 4 rows, 32 cols] into padded grid
                dst = xtp[ct][:, 0:PHW].rearrange("c (y x) -> c y x", x=PW)
                dst = dst[:, PAD + pt * 4: PAD + pt * 4 + 4, PAD:PAD + W]
                nc.scalar.activation(dst, ps[:].rearrange("c (r j) -> c r j", j=W),
                                     ACTF.Copy)

    # ---------- phase 2: packed 2x2-corner bf16 image ----------
    d2 = [d2_pool.tile([P, PHW, 4], BF16, tag=f"d2_{ct}", name=f"d2_{ct}") for ct in range(NCT)]
    for ct in range(NCT):
        for j, sh in enumerate((0, 1, PW, PW + 1)):
            nc.scalar.activation(d2[ct][:, :, j], xtp[ct][:, sh:sh + PHW], ACTF.Copy)

    # ---------- phase 3: offset conv ----------
    with tc.tile_pool(name="poff", bufs=1, space="PSUM") as poff_pool:
        ps_off = poff_pool.tile([OFFP, HW], F32)
        w_offT_v = w_offT[:].rearrange("r (k o) -> r k o", o=OFFP)
        n_k = NT * NCT
        for t in range(NT):
            ky, kx = t // 3, t % 3
            for ct in range(NCT):
                kt = t * NCT + ct
                rhs = xtp[ct][:, 0:PHW].rearrange("c (y x) -> c y x", x=PW)
                rhs = rhs[:, PAD - 1 + ky:PAD - 1 + ky + H, PAD - 1 + kx:PAD - 1 + kx + W]
                # N columns in rho-order: (p, y, xh) with pixel = y*32 + xh*16 + p
                rhs = rhs.rearrange("c y (xh p) -> c p y xh", p=16)
                for nh in range(2):
                    nc.tensor.matmul(
                        ps_off[:, nh * 512:(nh + 1) * 512],
                        w_offT_v[:, kt, :],
                        rhs[:, nh * 8:(nh + 1) * 8, :, :],
                        start=(kt == 0), stop=(kt == n_k - 1),
                    )
        nc.scalar.activation(off_s[:, 0:HW], ps_off[0:36, :], ACTF.Identity,
                             bias=b_off_s[:, 0:1])
        nc.scalar.activation(off_s[:, HW:], ps_off[XOFF:XOFF + 36, :], ACTF.Identity,
                             bias=b_off_s[:, 1:2])
    xtp_cm.__exit__(None, None, None)
    small_pool = ctx.enter_context(tc.tile_pool(name="small", bufs=1))

    # ---------- phase 4: indices and weights ----------
    # all tiles [36, 2048]: cols 0:1024 = y quantities, 1024:2048 = x (rho order)
    W2 = 2 * HW
    pos = small_pool.tile([36, W2], F32, tag="shareC", name="pos")
    nc.vector.tensor_tensor(pos[:], off_s[:], grid_s[:], op=ALU.add)
    rnd = small_pool.tile([36, W2], F32, tag="shareA", name="rnd")
    nc.vector.tensor_scalar(rnd[:], pos[:], MAGIC, None, op0=ALU.add)
    nc.vector.tensor_scalar(rnd[:], rnd[:], MAGIC, None, op0=ALU.subtract)
    cmp = small_pool.tile([36, W2], F32, tag="shareB", name="cmp")
    nc.vector.tensor_tensor(cmp[:], rnd[:], pos[:], op=ALU.is_gt)
    flr = small_pool.tile([36, W2], F32, tag="shareE", name="flr")
    nc.vector.tensor_tensor(flr[:], rnd[:], cmp[:], op=ALU.subtract)
    # clamp (y then x)
    nc.vector.tensor_scalar(flr[:, 0:HW], flr[:, 0:HW], 0.0, float(PH - 3), op0=ALU.max, op1=ALU.min)
    nc.vector.tensor_scalar(flr[:, HW:], flr[:, HW:], 0.0, float(PW - 2), op0=ALU.max, op1=ALU.min)
    frac = small_pool.tile([36, W2], F32)
    nc.vector.tensor_tensor(frac[:], pos[:], flr[:], op=ALU.subtract)

    qf = small_pool.tile([36, HW], F32, tag="shareA", name="qf")
    nc.vector.tensor_scalar(qf[:], flr[:, 0:HW], float(PW), None, op0=ALU.mult)
    nc.vector.tensor_tensor(qf[:], qf[:], flr[:, HW:], op=ALU.add)
    qi32 = small_pool.tile([36, HW], I32, tag="shareB", name="qi32")
    nc.vector.tensor_copy(qi32[:], qf[:])
    qi16a = small_pool.tile([36, HW], I16)
    nc.vector.tensor_copy(qi16a[:], qi32[:])

    # complement weights: gyx = 1 - frac  (dual-op tensor_scalar)
    gyx = small_pool.tile([36, W2], F32, name="gyx")
    nc.vector.tensor_scalar(gyx[:], frac[:], -1.0, 1.0, op0=ALU.mult, op1=ALU.add)
    fy = frac[:, 0:HW]
    fx = frac[:, HW:]
    gy = gyx[:, 0:HW]
    gx = gyx[:, HW:]
    wc = {}
    wc_tags = {"w00": "shareC", "w01": "shareB", "w10": "shareE", "w11": "w11"}
    for nm, a, b in (("w00", gy, gx), ("w01", gy, fx),
                     ("w10", fy, gx), ("w11", fy, fx)):
        wt = small_pool.tile([36, HW], BF16, tag=wc_tags[nm], name=nm)
        nc.vector.tensor_tensor(wt[:], a, b, op=ALU.mult)
        wc[nm] = wt

    # ---------- phase 5: wrap indices into per-core layout ----------
    # q is in rho-order, so the wrapped tile W[p, s] = q_rho[p*64+s] is a plain
    # reshape; replicate per group's 16-row blocks via a DRAM bounce.
    dram_pool = ctx.enter_context(tc.tile_pool(name="qdram", bufs=1, space="DRAM"))
    qa_dram = dram_pool.tile([36, HW], I16, name="qa_dram")
    nc.sync.dma_start(qa_dram[:], qi16a[:])
    qrep = dram_pool.tile([8, 36, HW], I16, name="qrep")
    for r in range(8):
        nc.sync.dma_start(qrep[r], qa_dram[:])
    for ct in range(NCT):
        for cb in range(8):
            g = (ct * 128 + cb * 16) // Cg
            dst = widx[cb * 16:(cb + 1) * 16,
                       ct * NT * 64:(ct + 1) * NT * 64]
            dst = dst.rearrange("p (t s) -> p t s", s=64)
            srcv = qrep[cb, gk_row(g, 0):gk_row(g, 0) + NT, :]
            srcv = srcv.rearrange("t (p s) -> p t s", p=16)
            nc.sync.dma_start(dst, srcv)

    # ---------- phase 6+7: sampling fused with dcn matmul, then BN+SiLU ----------
    ztags = ["shareE", "shareB", "shareC"]
    z = [small_pool.tile([P, HW], BF16, tag=ztags[m], name=f"z{m}") for m in range(NM)]
    w_dcnT_v = w_dcnT[:].rearrange("r (k o) -> r k o", o=DIM)
    corner_info = (("w00", 0), ("w01", 1), ("w10", 2), ("w11", 3))
    with tc.tile_pool(name="pacc", bufs=1, space="PSUM") as pacc_pool, \
         tc.tile_pool(name="pwt", bufs=1, space="PSUM") as pwt_pool:
        accs = [pacc_pool.tile([P, HW], F32, tag=f"pa{m}", name=f"pa{m}")
                for m in range(NM)]
        for t in range(NT):
            for ct in range(NCT):
                kt = t * NCT + ct
                gA = work_pool.tile([P, HW, 4], BF16, tag="gA", name="gA")
                wsl = widx[:, (ct * NT + t) * 64:(ct * NT + t + 1) * 64]
                nc.gpsimd.ap_gather(gA[:], d2[ct][:], wsl,
                                    channels=P, num_elems=PHW, d=4, num_idxs=HW)
                acc = work_pool.tile([P, HW], F32, tag="acc", name="acc")
                tmp = work_pool.tile([P, HW], F32, tag="tmp", name="tmp")
                for ci, (nm, jc) in enumerate(corner_info):
                    pw = pwt_pool.tile([P, HW], F32, tag="pw", name="pw")
                    wcv = wc[nm][:, :].rearrange("g (r q) -> g q r", q=64)
                    for nh in range(2):
                        nc.tensor.matmul(pw[:, nh * 512:(nh + 1) * 512],
                                         eoh_s[:, kt * P:(kt + 1) * P],
                                         wcv[:, nh * 32:(nh + 1) * 32, :],
                                         start=True, stop=True)
                    if ci == 0:
                        nc.vector.tensor_tensor(acc[:], gA[:, :, jc], pw[:], op=ALU.mult)
                    else:
                        nc.vector.tensor_tensor(tmp[:], gA[:, :, jc], pw[:], op=ALU.mult)
                        nc.vector.tensor_tensor(acc[:], acc[:], tmp[:], op=ALU.add)
                smp = samp_pool.tile([P, HW], BF16, tag="smp", name="smp", bufs=2)
                nc.scalar.activation(smp[:], acc[:], ACTF.Copy)
                for m in range(NM):
                    for nh in range(2):
                        nc.tensor.matmul(
                            accs[m][:, nh * 512:(nh + 1) * 512],
                            w_dcnT_v[:, kt, m * P:(m + 1) * P],
                            smp[:, nh * 512:(nh + 1) * 512],
                            start=(kt == 0), stop=(kt == NT * NCT - 1),
                        )
        for m in range(NM):
            zpre = work_pool.tile([P, HW], F32, tag="acc", name="zpre")
            zsig = work_pool.tile([P, HW], F32, tag="tmp", name="zsig")
            nc.scalar.activation(zpre[:], accs[m][:], ACTF.Identity,
                                 bias=bn_shift[:, m:m + 1], scale=bn_scale[:, m:m + 1])
            nc.scalar.activation(zsig[:], accs[m][:], ACTF.Sigmoid,
                                 bias=bn_shift[:, m:m + 1], scale=bn_scale[:, m:m + 1])
            nc.vector.tensor_tensor(z[m][:], zpre[:], zsig[:], op=ALU.mult)

    # ---------- phase 8: 1x1 conv + bias ----------
    ytags = ["w11", "gyx", "qi16a"]
    y = [small_pool.tile([P, HW], BF16, tag=ytags[m], name=f"yy{m}") for m in range(NM)]
    w2T_v = w2T[:].rearrange("r (k o) -> r k o", o=DIM)
    with tc.tile_pool(name="p2", bufs=3, space="PSUM") as p2_pool:
        for m in range(NM):
            ps = p2_pool.tile([P, HW], F32)
            for kt in range(NCT):
                for nh in range(2):
                    nc.tensor.matmul(
                        ps[:, nh * 512:(nh + 1) * 512],
                        w2T_v[:, kt, m * P:(m + 1) * P],
                        z[kt][:, nh * 512:(nh + 1) * 512],
                        start=(kt == 0), stop=(kt == NCT - 1),
                    )
            nc.scalar.activation(y[m][:], ps[:], ACTF.Identity, bias=bnv["b2"][:, m:m + 1])

    # ---------- phase 9: transpose out and store ----------
    with tc.tile_pool(name="pout", bufs=4, space="PSUM") as pout_pool:
        for pt in range(NPT):
            osb = small_pool.tile([P, DIM], F32, tag="frac", name="osb")
            for m in range(NM):
                ps = pout_pool.tile([P, P], BF16, name="pso")
                nc.tensor.transpose(ps[:], y[m][:, pt * P:(pt + 1) * P], ident_bf[:])
                nc.scalar.activation(osb[:, m * P:(m + 1) * P], ps[:], ACTF.Copy)
            nc.sync.dma_start(io["out"][pt * P:(pt + 1) * P, :], osb[:])


# ======================================================================
# SPMD entry point: full inputs in, full output out (8 cores, batch-parallel)
# ======================================================================

_PROGRAM_CACHE = {}


def _get_program(consts):
    key = "dcn"
    if key not in _PROGRAM_CACHE:
        import concourse.bacc as bacc
        nc = bacc.Bacc("TRN2", target_bir_lowering=False, debug=False)
        io = declare_io(nc, consts)
        with tile.TileContext(nc) as tc:
            with ExitStack() as ctx:
                build(ctx, tc, io)
        nc.compile()
        _PROGRAM_CACHE[key] = nc
    return _PROGRAM_CACHE[key]


def kernel(**inputs) -> np.ndarray:
    from concourse.bass_utils import run_bass_kernel_spmd

    x = np.ascontiguousarray(np.asarray(inputs["x"], np.float32))
    B = x.shape[0]
    assert x.shape == (B, HW, DIM), x.shape
    consts = host_prep(inputs)
    nc = _get_program(consts)
    n_cores = 8
    reps = []
    for i in range(n_cores):
        m = {"x": x[i % B]}
        m.update(consts)
        reps.append(m)
    res = run_bass_kernel_spmd(nc, reps, list(range(n_cores)))
    out = np.stack([np.asarray(res.results[i]["out"], np.float32)
                    for i in range(B)], axis=0)
    return out



# revision 7
# speedup vs baseline: 1.1210x; 1.1210x over previous
import os as _os
import sys as _sys

for _p in ("/opt/trn_rl_repo", "/root/.axon_site/_ro/trn_rl_repo",
           "/root/.axon_site", "/root/.axon_site/_ro/pypackages"):
    if _os.path.isdir(_p) and _p not in _sys.path:
        _sys.path.append(_p)

"""DCNv2 block kernel for TRN2 (Bass/Tile), v2.

Per-core program: one batch sample.
  x [1024, 384] -> transpose -> padded bf16 image xtp [384ch, 42*42]
  offset conv 3x3 (384->72, bf16) -> positions -> floor/frac/corner weights
  corner weights broadcast to channel partitions via DRAM-bounce DMA (bf16)
  ap_gather (d=4 packed corners, bf16) -> one TT mult + windowed reduce
  dcn matmul (K=3456, bf16) -> BN+SiLU (one Silu activation) -> 1x1 conv in
  pixel-major form (z as lhsT) -> out [1024, 384] with no output transposes.
"""

import numpy as np
from contextlib import ExitStack

import concourse.bass as bass
import concourse.tile as tile
from concourse import mybir
from concourse import library_config

F32 = mybir.dt.float32
BF16 = mybir.dt.bfloat16
I16 = mybir.dt.int16
I32 = mybir.dt.int32
ALU = mybir.AluOpType
ACTF = mybir.ActivationFunctionType

DIM, KK, G, Cg = 384, 9, 4, 96
H = W = 32
HW = 1024
PAD = 5
PH = PW = H + 2 * PAD          # 42
PHW = PH * PW                  # 1764
NT = KK                        # 9 taps
NCT = DIM // 128               # 3
NM = DIM // 128                # 3
OFFP = 100                     # offset rows: dy 0..35, dx 64..99
XOFF = 64
NPT = HW // 128                # 8
MAGIC = float(2 ** 23)

# (start, end, group) partition spans per channel tile
CT_SPANS = [
    [(0, 96, 0), (96, 128, 1)],
    [(0, 64, 1), (64, 128, 2)],
    [(0, 32, 2), (32, 128, 3)],
]


def gk_row(g, k):
    return g * KK + k


def host_prep(inputs: dict) -> dict:
    """Pure-layout host prep of weights/constants (shared by all cores)."""
    import ml_dtypes
    w_off = np.asarray(inputs["w_off"], np.float32)      # [72, 384, 3, 3]
    b_off = np.asarray(inputs["b_off"], np.float32)      # [72]
    w_dcn = np.asarray(inputs["w_dcn"], np.float32)      # [384, 384, 3, 3]
    w2 = np.asarray(inputs["w2"], np.float32)            # [384, 384]

    # offset conv rows: gk = dy rows 0..35, 36+gk = dx rows
    w_off_p = np.zeros((OFFP, DIM, 3, 3), np.float32)
    b_off_p = np.zeros((36, 2), np.float32)
    for g in range(G):
        for k in range(KK):
            w_off_p[gk_row(g, k)] = w_off[g * 18 + k * 2 + 0]
            w_off_p[XOFF + gk_row(g, k)] = w_off[g * 18 + k * 2 + 1]
            b_off_p[gk_row(g, k), 0] = b_off[g * 18 + k * 2 + 0]
            b_off_p[gk_row(g, k), 1] = b_off[g * 18 + k * 2 + 1]

    # offset conv lhsT tiles [128, 27, 72] bf16; K order = (tap, ct)
    w_offT = np.zeros((128, NT * NCT, OFFP), np.float32)
    for t in range(NT):
        ky, kx = t // 3, t % 3
        for ct in range(NCT):
            cs = ct * 128
            w_offT[:, t * NCT + ct, :] = w_off_p[:, cs:cs + 128, ky, kx].T
    w_offT = w_offT.reshape(128, NT * NCT * OFFP)

    # grid [36, 2*HW] f32, cols in rho order (rho(n) = (n%64)*16 + n//64)
    jj = np.arange(HW)
    rho = (jj % 64) * 16 + jj // 64
    yy = (np.arange(HW) // W)[rho]
    xx = (np.arange(HW) % W)[rho]
    grid_s = np.zeros((36, 2 * HW), np.float32)
    for g in range(G):
        for k in range(KK):
            grid_s[gk_row(g, k), 0:HW] = (k // 3 - 1) + yy + PAD
            grid_s[gk_row(g, k), HW:] = (k % 3 - 1) + xx + PAD

    # dcn lhsT tiles [128, 27, 384] bf16
    w_dcn_r = w_dcn.reshape(DIM, DIM, KK)
    w_dcnT = np.zeros((128, NT * NCT, DIM), np.float32)
    for t in range(NT):
        for ct in range(NCT):
            cs = ct * 128
            w_dcnT[:, t * NCT + ct, :] = w_dcn_r[:, cs:cs + 128, t].T
    w_dcnT = w_dcnT.reshape(128, NT * NCT * DIM)

    # 1x1 conv rhs tiles (pixel-major matmul): w2r[c, kt*384+o] = w2[o, kt*128+c]
    w2r = np.zeros((128, NCT, DIM), np.float32)
    for kt in range(NCT):
        w2r[:, kt, :] = w2[:, kt * 128:(kt + 1) * 128].T
    w2r = w2r.reshape(128, NCT * DIM)

    consts = {
        "w_offT": w_offT.astype(ml_dtypes.bfloat16),
        "b_off_p": b_off_p,
        "grid_s": grid_s,
        "w_dcnT": w_dcnT.astype(ml_dtypes.bfloat16),
        "w2r": w2r.astype(ml_dtypes.bfloat16),
        "b2r": np.asarray(inputs["b2"], np.float32).reshape(1, DIM).astype(ml_dtypes.bfloat16),
        "ones1": np.ones((1, 128), np.float32).astype(ml_dtypes.bfloat16),
        "ident": np.eye(128, dtype=np.float32),
        "sconst": np.tile(np.array([[MAGIC, -MAGIC, float(PW), 1.0, -1.0]],
                                   np.float32), (36, 1)),
        "bn_gamma": np.asarray(inputs["bn_gamma"], np.float32),
        "bn_beta": np.asarray(inputs["bn_beta"], np.float32),
        "bn_mean": np.asarray(inputs["bn_mean"], np.float32),
        "bn_var": np.asarray(inputs["bn_var"], np.float32),
    }
    return consts


def declare_io(nc: bass.Bass, consts: dict):
    aps = {}
    aps["x"] = nc.dram_tensor("x", [HW, DIM], F32, kind="ExternalInput").ap()
    for name, arr in consts.items():
        dt = {np.dtype("float32"): F32}.get(arr.dtype, BF16)
        aps[name] = nc.dram_tensor(name, list(arr.shape), dt, kind="ExternalInput").ap()
    aps["out"] = nc.dram_tensor("out", [HW, DIM], F32, kind="ExternalOutput").ap()
    return aps


def build(ctx: ExitStack, tc: tile.TileContext, io: dict):
    nc = tc.nc
    P = 128
    nc.gpsimd.load_library(library_config.ap_gather)

    const_pool = ctx.enter_context(tc.tile_pool(name="consts", bufs=1))
    d2_pool = ctx.enter_context(tc.tile_pool(name="d2", bufs=1))
    mid_pool = ctx.enter_context(tc.tile_pool(name="mid", bufs=1))

    # ---------- constants ----------
    w_offT = const_pool.tile([P, NT * NCT * OFFP], BF16)
    nc.sync.dma_start(w_offT[:], io["w_offT"][:])
    grid_s = const_pool.tile([36, 2 * HW], F32)
    nc.sync.dma_start(grid_s[:], io["grid_s"][:])
    w_dcnT = const_pool.tile([P, NT * NCT * DIM], BF16)
    nc.sync.dma_start(w_dcnT[:], io["w_dcnT"][:])
    w2r = const_pool.tile([P, NCT * DIM], BF16)
    nc.sync.dma_start(w2r[:], io["w2r"][:])
    b2r = const_pool.tile([1, DIM], BF16)
    nc.sync.dma_start(b2r[:], io["b2r"][:])
    ones1 = const_pool.tile([1, P], BF16)
    nc.sync.dma_start(ones1[:], io["ones1"][:])
    ident = const_pool.tile([P, P], F32)
    nc.sync.dma_start(ident[:], io["ident"][:])
    b_off_s = const_pool.tile([36, 2], F32)
    nc.sync.dma_start(b_off_s[:], io["b_off_p"][:])
    sconst = const_pool.tile([36, 5], F32)
    nc.sync.dma_start(sconst[:], io["sconst"][:])

    bnv = {}
    for vname in ("bn_gamma", "bn_beta", "bn_mean", "bn_var"):
        tl = const_pool.tile([P, NM], F32, tag=f"bn_{vname}", name=f"bn_{vname}")
        for m in range(NM):
            nc.sync.dma_start(
                tl[:, m:m + 1],
                io[vname][:].rearrange("(m p u) -> m p u", p=P, u=1)[m],
            )
        bnv[vname] = tl
    bn_scale = const_pool.tile([P, NM], F32)
    bn_shift = const_pool.tile([P, NM], F32)
    tmpv = const_pool.tile([P, NM], F32)
    nc.vector.tensor_scalar(tmpv[:], bnv["bn_var"][:], 1e-5, None, op0=ALU.add)
    nc.scalar.sqrt(tmpv[:], tmpv[:])
    nc.vector.reciprocal(tmpv[:], tmpv[:])
    nc.vector.tensor_tensor(bn_scale[:], bnv["bn_gamma"][:], tmpv[:], op=ALU.mult)
    nc.vector.tensor_tensor(tmpv[:], bnv["bn_mean"][:], bn_scale[:], op=ALU.mult)
    nc.vector.tensor_tensor(bn_shift[:], bnv["bn_beta"][:], tmpv[:], op=ALU.subtract)

    # ---------- phase 1: load x, transpose into padded bf16 image ----------
    xtp_cm = tc.tile_pool(name="xtp", bufs=1)
    xtp_pool = xtp_cm.__enter__()
    XTW = PHW + PW + 2
    xtp = [xtp_pool.tile([P, XTW], BF16, tag=f"xtp{ct}", name=f"xtp{ct}") for ct in range(NCT)]
    for ct in range(NCT):
        nc.vector.memset(xtp[ct][:], 0.0)

    with tc.tile_pool(name="ptrans", bufs=4, space="PSUM") as psum_t, \
         tc.tile_pool(name="xin", bufs=3) as xin_pool:
        for pt in range(NPT):
            xin = xin_pool.tile([P, DIM], F32)
            nc.sync.dma_start(xin[:], io["x"][pt * P:(pt + 1) * P, :])
            for ct in range(NCT):
                ps = psum_t.tile([P, P], F32)
                nc.tensor.transpose(ps[:], xin[:, ct * P:(ct + 1) * P], ident[:])
                dst = xtp[ct][:, 0:PHW].rearrange("c (y x) -> c y x", x=PW)
                dst = dst[:, PAD + pt * 4: PAD + pt * 4 + 4, PAD:PAD + W]
                nc.scalar.activation(dst, ps[:].rearrange("c (r j) -> c r j", j=W),
                                     ACTF.Copy)

    # ---------- phase 2: packed 4-corner bf16 image (split scalar/vector) ----------
    d2 = [d2_pool.tile([P, PHW, 4], BF16, tag=f"d2_{ct}", name=f"d2_{ct}") for ct in range(NCT)]
    for ct in range(NCT):
        for j, sh in enumerate((0, 1, PW, PW + 1)):
            if j % 2 == 0:
                nc.scalar.activation(d2[ct][:, :, j], xtp[ct][:, sh:sh + PHW], ACTF.Copy)
            else:
                nc.vector.tensor_copy(d2[ct][:, :, j], xtp[ct][:, sh:sh + PHW])

    # ---------- phase 3: offset conv (bf16) ----------
    off_s = mid_pool.tile([36, 2 * HW], F32, name="off_s")
    with tc.tile_pool(name="poff", bufs=1, space="PSUM") as poff_pool:
        ps_off = poff_pool.tile([OFFP, HW], F32)
        w_offT_v = w_offT[:].rearrange("r (k o) -> r k o", o=OFFP)
        n_k = NT * NCT
        for t in range(NT):
            ky, kx = t // 3, t % 3
            for ct in range(NCT):
                kt = t * NCT + ct
                rhs = xtp[ct][:, 0:PHW].rearrange("c (y x) -> c y x", x=PW)
                rhs = rhs[:, PAD - 1 + ky:PAD - 1 + ky + H, PAD - 1 + kx:PAD - 1 + kx + W]
                rhs = rhs.rearrange("c y (xh p) -> c p y xh", p=16)
                for nh in range(2):
                    nc.tensor.matmul(
                        ps_off[:, nh * 512:(nh + 1) * 512],
                        w_offT_v[:, kt, :],
                        rhs[:, nh * 8:(nh + 1) * 8, :, :],
                        start=(kt == 0), stop=(kt == n_k - 1),
                    )
        nc.scalar.activation(off_s[:, 0:HW], ps_off[0:36, :], ACTF.Identity,
                             bias=b_off_s[:, 0:1])
        nc.scalar.activation(off_s[:, HW:], ps_off[XOFF:XOFF + 36, :], ACTF.Identity,
                             bias=b_off_s[:, 1:2])
    xtp_cm.__exit__(None, None, None)

    # ---------- phase 4: positions, indices ----------
    W2 = 2 * HW
    small_cm = tc.tile_pool(name="small", bufs=1)
    small_pool = small_cm.__enter__()
    pos = small_pool.tile([36, W2], F32, name="pos")
    nc.vector.tensor_tensor(pos[:], off_s[:], grid_s[:], op=ALU.add)
    rnd = small_pool.tile([36, W2], F32, name="rnd")
    nc.scalar.add(rnd[:], pos[:], sconst[:, 0:1])
    nc.scalar.add(rnd[:], rnd[:], sconst[:, 1:2])
    cmp = small_pool.tile([36, W2], F32, name="cmp")
    nc.vector.tensor_tensor(cmp[:], rnd[:], pos[:], op=ALU.is_gt)
    flr = small_pool.tile([36, W2], F32, name="flr")
    nc.vector.tensor_tensor(flr[:], rnd[:], cmp[:], op=ALU.subtract)
    nc.vector.tensor_scalar(flr[:, 0:HW], flr[:, 0:HW], 0.0, float(PH - 2),
                            op0=ALU.max, op1=ALU.min)
    nc.vector.tensor_scalar(flr[:, HW:], flr[:, HW:], 0.0, float(PW - 2),
                            op0=ALU.max, op1=ALU.min)

    # indices first (unblocks phase 5 / gathers early)
    qf = small_pool.tile([36, HW], F32, name="qf")
    nc.scalar.mul(qf[:], flr[:, 0:HW], sconst[:, 2:3])
    nc.vector.tensor_tensor(qf[:], qf[:], flr[:, HW:], op=ALU.add)
    qi32 = small_pool.tile([36, HW], I32, name="qi32")
    nc.vector.tensor_copy(qi32[:], qf[:])
    qi16 = small_pool.tile([36, HW], I16, name="qi16")
    nc.vector.tensor_copy(qi16[:], qi32[:])

    # ---------- phase 5a: wrap indices via DRAM bounce ----------
    dram_pool = ctx.enter_context(tc.tile_pool(name="qdram", bufs=1, space="DRAM"))
    qa_dram = dram_pool.tile([36, HW], I16, name="qa_dram")
    nc.sync.dma_start(qa_dram[:], qi16[:])
    widx = [mid_pool.tile([P, NT * 64], I16, tag=f"widx{ct}", name=f"widx{ct}")
            for ct in range(NCT)]
    for ct in range(NCT):
        for cb in range(8):
            g = (ct * 128 + cb * 16) // Cg
            dst = widx[ct][cb * 16:(cb + 1) * 16, :].rearrange("p (t s) -> p t s", s=64)
            srcv = qa_dram[gk_row(g, 0):gk_row(g, 0) + NT, :]
            srcv = srcv.rearrange("t (p s) -> p t s", p=16)
            nc.sync.dma_start(dst, srcv)

    # ---------- phase 4b: corner weights, packed in gather output order ----------
    frac = small_pool.tile([36, W2], F32, name="frac")
    nc.vector.tensor_tensor(frac[:], pos[:], flr[:], op=ALU.subtract)
    gyx = small_pool.tile([36, W2], F32, name="gyx")
    nc.scalar.activation(gyx[:], frac[:], ACTF.Identity,
                         bias=sconst[:, 3:4], scale=sconst[:, 4:5])
    # wc_packed[g, m, j]: weights for gather-output col m (m-order); the
    # source cols are n-ordered with n = (m%16)*64 + m//16.
    wc_packed = small_pool.tile([36, HW * 4], BF16, name="wc_packed")
    wcp_v = wc_packed[:].rearrange("g (r s j) -> g s r j", r=64, s=16, j=4)
    fy = frac[:, 0:HW].rearrange("g (s r) -> g s r", s=16)
    fx = frac[:, HW:].rearrange("g (s r) -> g s r", s=16)
    gy = gyx[:, 0:HW].rearrange("g (s r) -> g s r", s=16)
    gx = gyx[:, HW:].rearrange("g (s r) -> g s r", s=16)
    nc.vector.tensor_tensor(wcp_v[:, :, :, 0], gy, gx, op=ALU.mult)
    nc.vector.tensor_tensor(wcp_v[:, :, :, 1], gy, fx, op=ALU.mult)
    nc.vector.tensor_tensor(wcp_v[:, :, :, 2], fy, gx, op=ALU.mult)
    nc.vector.tensor_tensor(wcp_v[:, :, :, 3], fy, fx, op=ALU.mult)

    # ---------- phase 5b: weight table to DRAM for broadcast ----------
    wc_dram = dram_pool.tile([36, HW * 4], BF16, name="wc_dram")
    nc.sync.dma_start(wc_dram[:], wc_packed[:])
    small_cm.__exit__(None, None, None)

    # ---------- phase 6+7: gather, weight, reduce, dcn matmul ----------
    w_dcnT_v = w_dcnT[:].rearrange("r (k o) -> r k o", o=DIM)
    z = [mid_pool.tile([P, HW], BF16, tag=f"z{m}", name=f"z{m}") for m in range(NM)]
    with tc.tile_pool(name="pacc", bufs=1, space="PSUM") as pacc_pool, \
         tc.tile_pool(name="gaP", bufs=3) as ga_pool, \
         tc.tile_pool(name="pwP", bufs=3) as pw_pool, \
         tc.tile_pool(name="prP", bufs=2) as pr_pool, \
         tc.tile_pool(name="smpP", bufs=3) as smp_pool:
        accs = [pacc_pool.tile([P, HW], F32, tag=f"pa{m}", name=f"pa{m}")
                for m in range(NM)]
        for t in range(NT):
            for ct in range(NCT):
                kt = t * NCT + ct
                pw = pw_pool.tile([P, HW * 4], BF16, tag="pw", name="pw")
                for (p0, p1, g) in CT_SPANS[ct]:
                    nc.scalar.dma_start(
                        pw[p0:p1, :],
                        wc_dram[gk_row(g, t):gk_row(g, t) + 1, :].broadcast_to(
                            [p1 - p0, HW * 4]),
                    )
                gA = ga_pool.tile([P, HW, 4], BF16, tag="gA", name="gA")
                wsl = widx[ct][:].rearrange("p (t s) -> p t s", s=64)[:, t, :]
                nc.gpsimd.ap_gather(gA[:], d2[ct][:], wsl,
                                    channels=P, num_elems=PHW, d=4, num_idxs=HW)
                prod = pr_pool.tile([P, HW * 4], BF16, tag="prod", name="prod")
                nc.vector.tensor_tensor(
                    prod[:], gA[:].rearrange("c m j -> c (m j)"), pw[:], op=ALU.mult)
                smp = smp_pool.tile([P, HW], BF16, tag="smp", name="smp")
                with nc.allow_low_precision("bf16 corner-sum feeds bf16 matmul"):
                    nc.vector.tensor_reduce(
                        smp[:], prod[:].rearrange("c (m j) -> c m j", j=4),
                        axis=mybir.AxisListType.X, op=ALU.add)
                for m in range(NM):
                    for nh in range(2):
                        nc.tensor.matmul(
                            accs[m][:, nh * 512:(nh + 1) * 512],
                            w_dcnT_v[:, kt, m * P:(m + 1) * P],
                            smp[:, nh * 512:(nh + 1) * 512],
                            start=(kt == 0), stop=(kt == NT * NCT - 1),
                        )
        # BN + SiLU in one activation per output tile
        for m in range(NM):
            nc.scalar.activation(z[m][:], accs[m][:], ACTF.Silu,
                                 bias=bn_shift[:, m:m + 1], scale=bn_scale[:, m:m + 1])

    # ---------- phase 8: 1x1 conv, pixel-major (output needs no transpose) ----------
    w2r_v = w2r[:].rearrange("c (k o) -> c k o", o=DIM)
    with tc.tile_pool(name="p8", bufs=3, space="PSUM") as p8_pool, \
         tc.tile_pool(name="osb", bufs=3) as osb_pool:
        for pt in range(NPT):
            ps = p8_pool.tile([P, DIM], F32)
            for kt in range(NCT):
                nc.tensor.matmul(
                    ps[:], z[kt][:, pt * P:(pt + 1) * P], w2r_v[:, kt, :],
                    start=(kt == 0), stop=False,
                )
            nc.tensor.matmul(ps[:], ones1[0:1, :], b2r[0:1, :],
                             start=False, stop=True)
            osb = osb_pool.tile([P, DIM], F32, tag="osb", name="osb")
            nc.scalar.activation(osb[:], ps[:], ACTF.Copy)
            nc.sync.dma_start(io["out"][pt * P:(pt + 1) * P, :], osb[:])


# ======================================================================
# SPMD entry point: full inputs in, full output out (8 cores, batch-parallel)
# ======================================================================

_PROGRAM_CACHE = {}


def _get_program(consts):
    key = "dcn2"
    if key not in _PROGRAM_CACHE:
        import concourse.bacc as bacc
        nc = bacc.Bacc("TRN2", target_bir_lowering=False, debug=False)
        io = declare_io(nc, consts)
        with tile.TileContext(nc) as tc:
            with ExitStack() as ctx:
                build(ctx, tc, io)
        nc.compile()
        _PROGRAM_CACHE[key] = nc
    return _PROGRAM_CACHE[key]


def kernel(**inputs) -> np.ndarray:
    from concourse.bass_utils import run_bass_kernel_spmd

    x = np.ascontiguousarray(np.asarray(inputs["x"], np.float32))
    B = x.shape[0]
    assert x.shape == (B, HW, DIM), x.shape
    consts = host_prep(inputs)
    nc = _get_program(consts)
    n_cores = 8
    reps = []
    for i in range(n_cores):
        m = {"x": x[i % B]}
        m.update(consts)
        reps.append(m)
    res = run_bass_kernel_spmd(nc, reps, list(range(n_cores)))
    out = np.stack([np.asarray(res.results[i]["out"], np.float32)
                    for i in range(B)], axis=0)
    return out
